# revision 1
# baseline (speedup 1.0000x reference)
"""Trainium2 Bass kernel for nn_BGNLLLoss (bivariate-Gaussian NLL loss).

Math (per element t,p):
    mux,muy,lsx,lsy,pc = params[t,p,:];  x,y = targets[t,p,:]
    sx=e^lsx, sy=e^lsy, c=tanh(pc), nr=1-c^2
    a=(x-mux)/sx, b=(y-muy)/sy
    nll = min( (a^2+b^2-2abc)/(2nr) + lsx+lsy + 0.5 ln(nr) + ln(2pi),
               -ln(1e-20) )
    loss[p] = sum_t nll[t,p]

tanh-free reformulation (keeps ScalarE in ONE table set: exp+ln+square):
  t4  = e^{-2 pc}            =>  c = (1-t4)/(1+t4),  nr = 4 t4/(1+t4)^2
  gv  = a(1+t4) + b(t4-1)    =  (a - cb)(1+t4)
  (a^2+b^2-2abc)/(2nr)       =  gv^2 e^{2pc}/8 + b^2/2
  0.5 ln(nr)                 =  ln2 - pc - ln(1+t4)
  nll = min( (gv st)^2 + bh^2 + (lsx+lsy-pc) - lvc, K )
    with st = e^{pc}/(2 sqrt2), bh = b/sqrt2,
         lvc = ln(1+t4) - (ln2 + ln 2pi)  [folded into the Ln's scale/bias]

Engine split (per 256-row block; all 16 blocks pipelined by Tile):
  ScalarE: isx, isyh(=isy/sqrt2), t4, st, lvc          (5 ACTIVATEs)
  GpSimd : ny, s1=lsx+lsy, s1b=s1-pc                   (3 tensor ops)
  VectorE: bf16 2x chain a,bh,av,qn,gv,gvs,u,b2,W,V + 2 ts + 1 custom min
  TensorE: frame sum   acc[1,512] += ones^T @ nll      (2 matmuls)
Sharding: person dim split across 8 cores (512 each), no collectives.
"""

import json
import math
import os
import shutil
import tempfile
from contextlib import ExitStack

import numpy as np

import concourse.bass as bass
import concourse.bacc as bacc
import concourse.mybir as mybir
import concourse.tile as tile
from concourse import bass_utils
from concourse.dve_spec import Spec, Src0, Src1, C0, C1, lower, sq, minn, _has_src1
from concourse.dve_uop import DveOpSpec
import concourse.dve_ops as dve_ops

F32 = mybir.dt.float32
BF16 = mybir.dt.bfloat16
AF = mybir.ActivationFunctionType
ALU = mybir.AluOpType

T = 4096
P = 4096
N_CORES = 8
PC = P // N_CORES          # persons per core = 512
K = 2                      # 128-row subtiles per block
RB = 128 * K               # rows per block
NB = T // RB               # 16 blocks
TGT_W = PC * 2             # 1024
PRM_W = PC * 5             # 2560

LOG2PI = math.log(2.0 * math.pi)
LN2 = math.log(2.0)
CADD = LN2 + LOG2PI                    # additive const inside the min
CLAMP = -math.log(1e-20)               # 46.0517...
SQRT2 = math.sqrt(2.0)
B_ISYH = -0.5 * LN2                    # exp bias: isy/sqrt(2)
B_ST = -1.5 * LN2                      # exp bias: e^{pc}/(2 sqrt 2)
SC_LN = math.exp(-CADD)                # ln scale/bias: ln(1+t4) - CADD


# --------------------------------------------------------------------------
# Custom DVE op: out = min(in0 + in1 + s0, s1)
# --------------------------------------------------------------------------
def _register_dve_op(name: str, spec: Spec, subdim: bool = False):
    if name in dve_ops._SUB_OPCODE_FOR_NAME:
        return next(op for op in dve_ops.OPS if op.name == name)
    shas = {}
    for ver in ("v3", "v4"):
        uops = lower(spec, ver=ver)
        shas[ver] = DveOpSpec(
            name=name, opcode=0, uops=uops, rd1_en=_has_src1(spec)
        ).sha(ver)
    op = dve_ops.DveOp(name, spec, subdim=subdim, uops_sha=shas)
    dve_ops.OPS.append(op)
    dve_ops._SUB_OPCODE_FOR_NAME[name] = (
        dve_ops._CUSTOM_DVE_ROW_BASE + len(dve_ops.OPS) - 1
    )
    dve_ops.CUSTOM_DVE_SPECS[name] = spec
    return op


ADDMIN = _register_dve_op(
    "ADDMIN_BGNLL",
    Spec(
        body=minn(Src0 + Src1 + C0, C1),
        reference=lambda in0, in1, s0, s1, imm2: np.minimum(
            in0.astype(np.float32) + in1 + s0, s1
        ).astype(np.float32),
    ),
)

# out = sq(in0) + sq(in1)
SQ2 = _register_dve_op(
    "SQ2_BGNLL",
    Spec(
        body=sq(Src0) + sq(Src1),
        reference=lambda in0, in1, s0, s1, imm2: (
            np.square(in0.astype(np.float32)) + np.square(in1.astype(np.float32))
        ).astype(np.float32),
    ),
)

# Fast-log constants: for x = 2^e (1+f), int_bits(x)/2^23 = e + 127 + f and
# log2(x) = e + log2(1+f), so ln(x) ~= (int_bits(x) - SIGMA) * ln2/2^23 with
# the mantissa correction c = E[log2(1+f) - f] = 1.5 - 1/ln2 (zero-mean over
# uniform f) and the additive constant CADD both folded into SIGMA.
LNK = math.log(2.0) / (1 << 23)
_C_MEAN = 1.5 - 1.0 / math.log(2.0)            # 0.0573049...
SIGMA_F = (127.0 - _C_MEAN + CADD / math.log(2.0)) * (1 << 23)



# --------------------------------------------------------------------------
# ACT table-set fix: walrus assigns Exp -> exp_and_others and Ln ->
# natural_log_exp_and_others, reloading tables every block (~2.6us/block).
# Reorder act_info.json so the combined exp+ln set is found first for both.
# --------------------------------------------------------------------------
def _install_act_json():
    if os.environ.get("BGNLL_NO_ACT_JSON"):
        return
    if os.environ.get("BASS_ACT_ROOT_JSON_PATH"):
        return
    try:
        from neuronxcc.driver.Job import Job
        from neuronxcc.driver.jobs.support.FindActInfo import findActInfoFile
        src = findActInfoFile(Job.getPackageDir(), "gen3")
    except Exception:
        return
    if not src:
        return
    src_dir = os.path.dirname(src)
    dst_dir = os.path.join(tempfile.gettempdir(), "bgnll_act_root")
    os.makedirs(dst_dir, exist_ok=True)
    with open(src) as f:
        info = json.load(f)
    sets = info.get("act_func_sets", [])
    pref = [s for s in sets if s.get("name") == "natural_log_exp_and_others"]
    rest = [s for s in sets if s.get("name") != "natural_log_exp_and_others"]
    if not pref:
        return
    info["act_func_sets"] = pref + rest
    for name in os.listdir(src_dir):
        s = os.path.join(src_dir, name)
        d = os.path.join(dst_dir, name)
        if os.path.isfile(s) and not os.path.exists(d) and name != "act_info.json":
            try:
                os.symlink(s, d)
            except OSError:
                shutil.copy(s, d)
    with open(os.path.join(dst_dir, "act_info.json"), "w") as f:
        json.dump(info, f)
    os.environ["BASS_ACT_ROOT_JSON_PATH"] = os.path.join(dst_dir, "act_info.json")


# --------------------------------------------------------------------------
# Kernel body (per core; SPMD -- same program on all 8 cores)
# --------------------------------------------------------------------------
def _emit(ctx: ExitStack, tc: tile.TileContext, tgt: bass.AP, prm: bass.AP,
          loss: bass.AP):
    nc = tc.nc

    iot = ctx.enter_context(tc.tile_pool(name="iot", bufs=3))
    iop = ctx.enter_context(tc.tile_pool(name="iop", bufs=4))
    tp = ctx.enter_context(tc.tile_pool(name="tp", bufs=3))
    tp2 = ctx.enter_context(tc.tile_pool(name="tp2", bufs=2))
    single = ctx.enter_context(tc.tile_pool(name="single", bufs=1))
    psum_pool = ctx.enter_context(
        tc.tile_pool(name="psum", bufs=1, space="PSUM")
    )

    ones = single.tile([128, 1], F32)
    nc.vector.memset(ones[:], 1.0)
    acc = psum_pool.tile([1, PC], F32)

    shb = [128, K, PC]
    ctxs: dict[int, dict] = {}

    def stage_load(blk):
        r0 = blk * RB
        tgv = tgt[r0:r0 + RB, :].rearrange("(k p) w -> p k w", k=K, p=128)
        prv = prm[r0:r0 + RB, :].rearrange("(k p) w -> p k w", k=K, p=128)
        tg = iot.tile([128, K, TGT_W], F32, tag="tg")
        nc.sync.dma_start(tg[:], tgv)
        pr = iop.tile([128, K, PRM_W], F32, tag="pr")
        nc.sync.dma_start(pr[:], prv)
        ctxs[blk] = {"tg": tg, "pr": pr}

    def stage_front(blk):
        c = ctxs[blk]
        tg4 = c["tg"][:].rearrange("p k (n c) -> p k n c", c=2)
        pr4 = c["pr"][:].rearrange("p k (n c) -> p k n c", c=5)
        c["t0v"], c["t1v"] = tg4[:, :, :, 0], tg4[:, :, :, 1]
        c["p0v"], c["p1v"] = pr4[:, :, :, 0], pr4[:, :, :, 1]
        p2v, p3v, p4v = pr4[:, :, :, 2], pr4[:, :, :, 3], pr4[:, :, :, 4]
        c["p2v"], c["p3v"], c["p4v"] = p2v, p3v, p4v

        t4 = tp.tile(shb, F32, tag="t4")
        t4p1f = tp.tile(shb, F32, tag="t4p1f")
        t4m1s = tp.tile(shb, BF16, tag="t4m1s")
        isx = tp.tile(shb, BF16, tag="isx")
        isyh = tp.tile(shb, BF16, tag="isyh")
        st = tp.tile(shb, BF16, tag="st")
        lvc = tp.tile(shb, BF16, tag="lvc")
        B = tp.tile(shb, BF16, tag="B")      # nyt -> bh
        S = tp.tile(shb, F32, tag="S")       # s1 -> s1b
        c.update(t4=t4, t4p1f=t4p1f, t4m1s=t4m1s, isx=isx, isyh=isyh,
                 st=st, lvc=lvc, B=B, S=S)

        # --- ScalarE: Exp-only (single table set) + affines ---
        nc.scalar.activation(t4[:], p4v, AF.Exp, scale=-2.0)
        nc.scalar.activation(t4p1f[:], t4[:], AF.Identity, scale=1.0,
                             bias=1.0)
        nc.scalar.activation(t4m1s[:], t4[:], AF.Identity, scale=SQRT2,
                             bias=-SQRT2)
        nc.scalar.activation(isx[:], p2v, AF.Exp, scale=-1.0)
        nc.scalar.activation(isyh[:], p3v, AF.Exp, scale=-1.0, bias=B_ISYH)
        nc.scalar.activation(st[:], p4v, AF.Exp, scale=1.0, bias=B_ST)
        # lvc = ln(1+t4) - CADD via the exponent-bits log approximation:
        # int32 bits of t4p1f, converted + affine-mapped in one ACTIVATE.
        nc.scalar.activation(lvc[:], t4p1f[:].bitcast(mybir.dt.int32),
                             AF.Identity, scale=LNK, bias=-SIGMA_F * LNK)

        # --- GpSimd: the fp32 strided side-chain ---
        nc.gpsimd.tensor_sub(B[:], c["t1v"], c["p1v"])        # nyt
        nc.gpsimd.tensor_add(S[:], p2v, p3v)                  # s1
        nc.gpsimd.tensor_sub(S[:], S[:], p4v)                 # s1b

    def stage_dve(blk):
        c = ctxs[blk]
        A = tp.tile(shb, BF16, tag="A")      # nxt -> a
        G = tp2.tile(shb, BF16, tag="G")     # av -> gv -> gvs
        qn = tp2.tile(shb, BF16, tag="qn")
        W = tp2.tile(shb, BF16, tag="W")
        VN = tp2.tile(shb, F32, tag="VN")    # V -> nll
        B, S = c["B"], c["S"]

        nc.vector.tensor_sub(A[:], c["t0v"], c["p0v"])        # nxt
        nc.vector.tensor_mul(A[:], A[:], c["isx"][:])         # a
        nc.vector.tensor_mul(B[:], B[:], c["isyh"][:])        # bh
        nc.vector.tensor_mul(G[:], A[:], c["t4p1f"][:])       # av
        nc.vector.tensor_mul(qn[:], B[:], c["t4m1s"][:])
        nc.vector.tensor_add(G[:], G[:], qn[:])               # gv
        nc.vector.tensor_mul(G[:], G[:], c["st"][:])          # gvs
        Wf = W[:].rearrange("p k n -> p (k n)")
        nc.vector._custom_dve(SQ2, out=Wf,
                              in0=G[:].rearrange("p k n -> p (k n)"),
                              in1=B[:].rearrange("p k n -> p (k n)"))
        nc.vector.tensor_sub(VN[:], W[:], c["lvc"][:])        # V
        Vf = VN[:].rearrange("p k n -> p (k n)")
        Sf = S[:].rearrange("p k n -> p (k n)")
        nc.vector._custom_dve(ADDMIN, out=Vf, in0=Vf, in1=Sf, s0=0.0,
                              s1=CLAMP)

        # --- TensorE: frame sum ---
        for k in range(K):
            nc.tensor.matmul(
                acc[:, :], ones[:, :], VN[:, k, :],
                start=(blk == 0 and k == 0),
                stop=(blk == NB - 1 and k == K - 1),
            )
        del ctxs[blk]

    # Skewed emission (software pipelining): DMA for blk+2, producers for
    # blk+1, consumers for blk — gives the static scheduler cross-block
    # interleaving priority.
    for i in range(NB + 2):
        if i < NB:
            stage_load(i)
        if 1 <= i and i - 1 < NB:
            stage_front(i - 1)
        if 2 <= i and i - 2 < NB:
            stage_dve(i - 2)

    out_sb = single.tile([1, PC], F32)
    nc.vector.tensor_copy(out_sb[:], acc[:, :])
    nc.sync.dma_start(loss, out_sb[:])


_CACHED_NC = None


def _build_program() -> bass.Bass:
    global _CACHED_NC
    if _CACHED_NC is not None:
        return _CACHED_NC
    nc = bacc.Bacc("TRN2", target_bir_lowering=False, debug=False,
                   enable_asserts=False)
    for v in (B_ISYH, B_ST, -SQRT2, -SIGMA_F * LNK):
        t = nc.alloc_sbuf_tensor(f"const-f32-{v}", [128, 1], F32)
        nc.gpsimd.memset(t.ap(), v)
        nc.const_aps.aps[(F32, v)] = t.ap()
    nc.all_engine_barrier()
    tgt = nc.dram_tensor("tgt", [T, TGT_W], F32, kind="ExternalInput").ap()
    prm = nc.dram_tensor("prm", [T, PRM_W], F32, kind="ExternalInput").ap()
    loss = nc.dram_tensor("loss", [1, PC], F32, kind="ExternalOutput").ap()
    with tile.TileContext(nc) as tc:
        with ExitStack() as ctx:
            _emit(ctx, tc, tgt, prm, loss)
    nc.compile()
    _CACHED_NC = nc
    return nc


def make_in_maps(targets: np.ndarray, params: np.ndarray):
    targets = np.asarray(targets, dtype=np.float32)
    params = np.asarray(params, dtype=np.float32)
    in_maps = []
    for i in range(N_CORES):
        sl = slice(i * PC, (i + 1) * PC)
        in_maps.append({
            "tgt": np.ascontiguousarray(targets[:, sl, :]).reshape(T, TGT_W),
            "prm": np.ascontiguousarray(params[:, sl, :]).reshape(T, PRM_W),
        })
    return in_maps


def run_spmd(targets: np.ndarray, params: np.ndarray, trace: bool = False):
    nc = _build_program()
    in_maps = make_in_maps(targets, params)
    res = bass_utils.run_bass_kernel_spmd(
        nc, in_maps, core_ids=list(range(N_CORES)), trace=trace,
    )
    loss = np.concatenate(
        [res.results[i]["loss"].reshape(PC) for i in range(N_CORES)]
    ).astype(np.float32)
    return loss, res


def kernel(targets: np.ndarray, params: np.ndarray,
           peopleIDs: np.ndarray | None = None) -> np.ndarray:
    loss, _ = run_spmd(targets, params, trace=False)
    return loss



# revision 3
# speedup vs baseline: 1.4082x; 1.4082x over previous
"""Trainium2 Bass kernel for nn_BGNLLLoss (bivariate-Gaussian NLL loss).

Math (per element t,p):
    mux,muy,lsx,lsy,pc = params[t,p,:];  x,y = targets[t,p,:]
    sx=e^lsx, sy=e^lsy, c=tanh(pc), nr=1-c^2
    a=(x-mux)/sx, b=(y-muy)/sy
    nll = min( (a^2+b^2-2abc)/(2nr) + lsx+lsy + 0.5 ln(nr) + ln(2pi),
               -ln(1e-20) )
    loss[p] = sum_t nll[t,p]

Reformulation used here (all engines stay in contiguous bf16):
  t4  = e^{-2 pc};  ah = (x-mux) e^{-lsx}/sqrt2;  bh = (y-muy) e^{-lsy}/sqrt2
  gs  = (ah+bh) t4 + (ah-bh);  gvs = gs e^{pc}/2      [= (a-cb)/sqrt(2nr)]
  W   = gvs^2 + bh^2                                  [= z/(2nr)]
  V   = W + (lsx+lsy-pc - ln(1+t4))                   [= nll - ln2 - ln2pi]
  nll = min(V + CADD, K) = K - relu((K-CADD) - V)
  loss[p] = T*K - sum_t relu((K-CADD) - V)            [relu'd row bounded ~50,
                                                       so bf16 sum is safe]

Input layout: host converts to bf16 planes and packs one contiguous
[128, 7ch * 4k * 512p] row block per DMA (28 KiB/partition).  Engine split
per 512-frame block (8 blocks, software-pipelined):
  ScalarE: t4, isxh, isyh, sth (Exp), lvc (Ln via free affine ln(t4+1)),
           r2 = Relu((K-CADD) - V)                    (6 ACTIVATEs)
  GpSimd : s1 = lsx+lsy; s1b = s1-pc; s1bl = s1b-lvc  (3 tensor ops)
  VectorE: 10 plain bf16 2x passes + 1 custom SQ2
  TensorE: frame sum   acc[1,512] += ones^T @ r2      (4 matmuls)
Sharding: person dim split across 8 cores (512 each), no collectives.
"""

import json
import math
import os
import shutil
import tempfile
from contextlib import ExitStack

import numpy as np
import ml_dtypes

import concourse.bass as bass
import concourse.bacc as bacc
import concourse.mybir as mybir
import concourse.tile as tile
from concourse import bass_utils
from concourse.dve_spec import Spec, Src0, Src1, sq, _has_src1
from concourse.dve_uop import DveOpSpec
import concourse.dve_ops as dve_ops

F32 = mybir.dt.float32
BF16 = mybir.dt.bfloat16
AF = mybir.ActivationFunctionType

T = 4096
P = 4096
N_CORES = 8
PC = P // N_CORES          # persons per core = 512
K = 4                      # 128-row subtiles per block
RB = 128 * K               # rows per block = 512
NB = T // RB               # 8 blocks
NCH = 7                    # x, y, mux, muy, lsx, lsy, pc
FD = K * PC                # free-dim elems per plane per block = 2048
ROW_W = NCH * FD           # bf16 elems per DMA row = 14336

LOG2PI = math.log(2.0 * math.pi)
LN2 = math.log(2.0)
CADD = LN2 + LOG2PI                    # nll = V + CADD before clamping
KCLAMP = -math.log(1e-20)              # 46.0517...
B_EXPH = -0.5 * LN2                    # exp bias: e^{-l}/sqrt2
B_STH = -LN2                           # exp bias: e^{pc}/2
B_LN1 = 1.0                            # ln bias: ln(t4 + 1)
B_RELU = KCLAMP - CADD                 # relu bias: relu(-V + (K-CADD))
TK_CONST = T * KCLAMP                  # loss = T*K - sum(r2)

BF_NP = ml_dtypes.bfloat16


# --------------------------------------------------------------------------
# Custom DVE op: out = sq(in0) + sq(in1)  (one pass for gvs^2 + bh^2)
# --------------------------------------------------------------------------
def _register_dve_op(name: str, spec: Spec, subdim: bool = False):
    if name in dve_ops._SUB_OPCODE_FOR_NAME:
        return next(op for op in dve_ops.OPS if op.name == name)
    shas = {}
    for ver in ("v3", "v4"):
        uops = dve_spec_lower(spec, ver=ver)
        shas[ver] = DveOpSpec(
            name=name, opcode=0, uops=uops, rd1_en=_has_src1(spec)
        ).sha(ver)
    op = dve_ops.DveOp(name, spec, subdim=subdim, uops_sha=shas)
    dve_ops.OPS.append(op)
    dve_ops._SUB_OPCODE_FOR_NAME[name] = (
        dve_ops._CUSTOM_DVE_ROW_BASE + len(dve_ops.OPS) - 1
    )
    dve_ops.CUSTOM_DVE_SPECS[name] = spec
    return op


from concourse.dve_spec import lower as dve_spec_lower

SQ2 = _register_dve_op(
    "SQ2_BGNLL",
    Spec(
        body=sq(Src0) + sq(Src1),
        reference=lambda in0, in1, s0, s1, imm2: (
            np.square(in0.astype(np.float32)) + np.square(in1.astype(np.float32))
        ).astype(np.float32),
    ),
)


# --------------------------------------------------------------------------
# ACT table-set fix: walrus assigns Exp -> exp_and_others and Ln ->
# natural_log_exp_and_others, reloading tables every block (~2.6us/block).
# Reorder act_info.json so the combined exp+ln set is found first for both.
# --------------------------------------------------------------------------
def _install_act_json():
    if os.environ.get("BGNLL_NO_ACT_JSON"):
        return
    if os.environ.get("BASS_ACT_ROOT_JSON_PATH"):
        return
    try:
        from neuronxcc.driver.Job import Job
        from neuronxcc.driver.jobs.support.FindActInfo import findActInfoFile
        src = findActInfoFile(Job.getPackageDir(), "gen3")
    except Exception:
        return
    if not src:
        return
    src_dir = os.path.dirname(src)
    dst_dir = os.path.join(tempfile.gettempdir(), "bgnll_act_root")
    os.makedirs(dst_dir, exist_ok=True)
    with open(src) as f:
        info = json.load(f)
    sets = info.get("act_func_sets", [])
    pref = [s for s in sets if s.get("name") == "natural_log_exp_and_others"]
    rest = [s for s in sets if s.get("name") != "natural_log_exp_and_others"]
    if not pref:
        return
    info["act_func_sets"] = pref + rest
    for name in os.listdir(src_dir):
        s = os.path.join(src_dir, name)
        d = os.path.join(dst_dir, name)
        if os.path.isfile(s) and not os.path.exists(d) and name != "act_info.json":
            try:
                os.symlink(s, d)
            except OSError:
                shutil.copy(s, d)
    with open(os.path.join(dst_dir, "act_info.json"), "w") as f:
        json.dump(info, f)
    os.environ["BASS_ACT_ROOT_JSON_PATH"] = os.path.join(dst_dir, "act_info.json")


# --------------------------------------------------------------------------
# Kernel body (per core; SPMD -- same program on all 8 cores)
# --------------------------------------------------------------------------
def _emit(ctx: ExitStack, tc: tile.TileContext, inp: bass.AP, loss: bass.AP):
    nc = tc.nc

    iot = ctx.enter_context(tc.tile_pool(name="iot", bufs=3))
    tp = ctx.enter_context(tc.tile_pool(name="tp", bufs=2))
    single = ctx.enter_context(tc.tile_pool(name="single", bufs=1))
    psum_pool = ctx.enter_context(
        tc.tile_pool(name="psum", bufs=1, space="PSUM")
    )

    ones = single.tile([128, 1], BF16)
    nc.vector.memset(ones[:], 1.0)
    acc = psum_pool.tile([1, PC], F32)

    shf = [128, FD]
    ctxs: dict[int, dict] = {}

    def stage_load(blk):
        tg = iot.tile([128, NCH, K, PC], BF16, tag="in")
        nc.sync.dma_start(tg[:].rearrange("p c k n -> p (c k n)"),
                          inp[blk * 128:(blk + 1) * 128, :])
        ctxs[blk] = {"in": tg}

    def chv(c, i):
        return c["in"][:, i].rearrange("p k n -> p (k n)")

    def stage_front(blk):
        c = ctxs[blk]
        xv, yv = chv(c, 0), chv(c, 1)
        mxv, myv = chv(c, 2), chv(c, 3)
        lxv, lyv, pcv = chv(c, 4), chv(c, 5), chv(c, 6)

        t4 = tp.tile(shf, BF16, tag="t4")
        isxh = tp.tile(shf, BF16, tag="isxh")
        isyh = tp.tile(shf, BF16, tag="isyh")
        sth = tp.tile(shf, BF16, tag="sth")
        lvc = tp.tile(shf, BF16, tag="lvc")
        S = tp.tile(shf, BF16, tag="S")
        c.update(t4=t4, isxh=isxh, isyh=isyh, sth=sth, lvc=lvc, S=S,
                 xv=xv, yv=yv, mxv=mxv, myv=myv)

        # --- ScalarE: one table set (exp + ln) ---
        nc.scalar.activation(t4[:], pcv, AF.Exp, scale=-2.0)
        nc.scalar.activation(isxh[:], lxv, AF.Exp, scale=-1.0, bias=B_EXPH)
        nc.scalar.activation(isyh[:], lyv, AF.Exp, scale=-1.0, bias=B_EXPH)
        nc.scalar.activation(sth[:], pcv, AF.Exp, scale=1.0, bias=B_STH)
        nc.scalar.activation(lvc[:], t4[:], AF.Ln, scale=1.0, bias=B_LN1)

        # --- GpSimd: the log-det side chain ---
        nc.gpsimd.tensor_add(S[:], lxv, lyv)          # s1 = lsx+lsy
        nc.gpsimd.tensor_sub(S[:], S[:], pcv)         # s1b = s1-pc
        nc.gpsimd.tensor_sub(S[:], S[:], lvc[:])      # s1bl = s1b-lvc

    def stage_dve(blk):
        c = ctxs[blk]
        A = tp.tile(shf, BF16, tag="A")      # nxt -> ah -> dab -> ...
        B = tp.tile(shf, BF16, tag="B")      # nyt -> bh
        C = tp.tile(shf, BF16, tag="C")      # sab -> m1 -> gs -> gvs -> W -> V
        r2 = tp.tile([128, K, PC], BF16, tag="r2")

        nc.vector.tensor_sub(A[:], c["xv"], c["mxv"])         # nxt
        nc.vector.tensor_sub(B[:], c["yv"], c["myv"])         # nyt
        nc.vector.tensor_mul(A[:], A[:], c["isxh"][:])        # ah
        nc.vector.tensor_mul(B[:], B[:], c["isyh"][:])        # bh
        nc.vector.tensor_add(C[:], A[:], B[:])                # sab
        nc.vector.tensor_sub(A[:], A[:], B[:])                # dab
        nc.vector.tensor_mul(C[:], C[:], c["t4"][:])          # m1
        nc.vector.tensor_add(C[:], C[:], A[:])                # gs
        nc.vector.tensor_mul(C[:], C[:], c["sth"][:])         # gvs
        nc.vector._custom_dve(SQ2, out=C[:], in0=C[:], in1=B[:])   # W
        nc.vector.tensor_add(C[:], C[:], c["S"][:])           # V

        # r2 = relu((K-CADD) - V); nll = K - r2
        nc.scalar.activation(r2[:].rearrange("p k n -> p (k n)"), C[:],
                             AF.Relu, scale=-1.0, bias=B_RELU)

        # --- TensorE: frame sum of r2 ---
        for k in range(K):
            nc.tensor.matmul(
                acc[:, :], ones[:, :], r2[:, k, :],
                start=(blk == 0 and k == 0),
                stop=(blk == NB - 1 and k == K - 1),
            )
        del ctxs[blk]

    # Skewed emission (software pipelining): DMA for blk+2, producers for
    # blk+1, consumers for blk.
    for i in range(NB + 2):
        if i < NB:
            stage_load(i)
        if 1 <= i and i - 1 < NB:
            stage_front(i - 1)
        if 2 <= i and i - 2 < NB:
            stage_dve(i - 2)

    # loss = T*K - acc
    tk = single.tile([1, PC], F32)
    nc.vector.memset(tk[:], TK_CONST)
    out_sb = single.tile([1, PC], F32)
    nc.vector.tensor_sub(out_sb[:], tk[:], acc[:, :])
    nc.sync.dma_start(loss, out_sb[:])


_CACHED_NC = None


def _build_program() -> bass.Bass:
    global _CACHED_NC
    if _CACHED_NC is not None:
        return _CACHED_NC
    _install_act_json()
    nc = bacc.Bacc("TRN2", target_bir_lowering=False, debug=False,
                   enable_asserts=False)
    for v in (B_EXPH, B_STH, B_LN1, B_RELU):
        t = nc.alloc_sbuf_tensor(f"const-f32-{v}", [128, 1], F32)
        nc.gpsimd.memset(t.ap(), v)
        nc.const_aps.aps[(F32, v)] = t.ap()
    nc.all_engine_barrier()
    inp = nc.dram_tensor("inp", [NB * 128, ROW_W], BF16,
                         kind="ExternalInput").ap()
    loss = nc.dram_tensor("loss", [1, PC], F32, kind="ExternalOutput").ap()
    with tile.TileContext(nc) as tc:
        with ExitStack() as ctx:
            _emit(ctx, tc, inp, loss)
    nc.compile()
    _CACHED_NC = nc
    return nc


def make_in_maps(targets: np.ndarray, params: np.ndarray):
    targets = np.asarray(targets, dtype=np.float32)
    params = np.asarray(params, dtype=np.float32)
    in_maps = []
    for i in range(N_CORES):
        sl = slice(i * PC, (i + 1) * PC)
        # planes: x, y, mux, muy, lsx, lsy, pc  -> [T, NCH, PC] bf16
        pl = np.concatenate(
            [targets[:, sl, :].transpose(0, 2, 1),
             params[:, sl, :].transpose(0, 2, 1)], axis=1
        ).astype(BF_NP)                          # [T, 7, 512]
        # row t = blk*512 + k*128 + p  ->  [NB, 128, 7, K, 512]
        pl = pl.reshape(NB, K, 128, NCH, PC).transpose(0, 2, 3, 1, 4)
        in_maps.append({
            "inp": np.ascontiguousarray(pl).reshape(NB * 128, ROW_W),
        })
    return in_maps


def run_spmd(targets: np.ndarray, params: np.ndarray, trace: bool = False):
    nc = _build_program()
    in_maps = make_in_maps(targets, params)
    res = bass_utils.run_bass_kernel_spmd(
        nc, in_maps, core_ids=list(range(N_CORES)), trace=trace,
    )
    loss = np.concatenate(
        [res.results[i]["loss"].reshape(PC) for i in range(N_CORES)]
    ).astype(np.float32)
    return loss, res


def kernel(targets: np.ndarray, params: np.ndarray,
           peopleIDs: np.ndarray | None = None) -> np.ndarray:
    loss, _ = run_spmd(targets, params, trace=False)
    return loss
